# revision 1
# baseline (speedup 1.0000x reference)
"""MoE LoRA linear layer kernel for Trainium2, data-parallel over 8 NeuronCores.

Math (per token n):
    down = h @ down_w.T                      [N, 64]
    mask[n, r] = val[n, k] if idx[n, k] == r else 0   (indices distinct per row)
    out = (down * mask) @ up_w.T             [N, 4096]

Sharding: tokens split 8 ways (2048/core); LoRA weights replicated.

Per-core pipeline (token tile TT=256 = 2 chunks of 128):
  1. load h in natural layout [128, 4096] per chunk (16KB DMA descriptors;
     a strided transpose-load would be 512B/descriptor and bottleneck the
     sync engine on descriptor generation)
  2. PE-transpose h blocks, 4 per PSUM bank, one fat [128, 512] copy each
     (copies alternate DVE/ACT)
  3. 32 f32r matmuls accumulate downT = dwT.T @ hT into PSUM [64, 256]
  4. top-k scatter mask: 8x tensor_scalar one-hot*val on DVE, transposed
     into one PSUM bank with matmul accumulation (no DVE adds), multiply
     with downT -> resT
  5. up-proj per chunk: 8x f32r matmul [K=64, M=128, N=512] -> psum,
     assemble out_sb [128, 4096], single fat store per chunk

f32r (4-byte storage, reduced-precision PE multiply) runs matmuls at 1
cycle/row for free dims >= 256 vs 4 cycles/row for plain fp32.

All small constants (dwT, identity, iota, idx, val) are host-packed into one
[128, CB] blob = single DMA.
"""

import sys

for p in ("/opt/trn_rl_repo", "/opt/pypackages"):
    if p not in sys.path:
        sys.path.insert(0, p)

import numpy as np

N, D_IN, D_OUT, RANK, TOPK = 16384, 4096, 4096, 64, 8
NCORES = 8
NT = N // NCORES          # tokens per core = 2048
P = 128                   # partitions
TT = 256                  # token tile (down-matmul free dim)
NKC = D_IN // P           # 32 contraction chunks for down proj
NJ = TT // P              # 2 x 128-token chunks per tile
NTILES = NT // TT         # 8 token tiles per core
NCHUNK = NT // P          # 16 x 128-token chunks per core
OT = 512                  # output col tile
NOT = D_OUT // OT         # 8 output col tiles

# const blob column layout (f32, [128, CB])
C_DWT = 0                 # [128, 32*64]   dwT chunk ki at C_DWT + ki*64
C_ID = C_DWT + NKC * RANK           # [128, 128] identity
C_IOTA = C_ID + P                   # [128, 64]  iota over rank
C_IDX = C_IOTA + RANK               # [128, 16*8] idx (chunk-major)
C_VAL = C_IDX + NCHUNK * TOPK       # [128, 16*8] val
CB = C_VAL + NCHUNK * TOPK

_CACHE = {}


def _build_program():
    import concourse.bacc as bacc
    import concourse.mybir as mybir
    from concourse import tile

    f32 = mybir.dt.float32
    f32r = mybir.dt.float32r
    # Bacc (not plain Bass): its finalize() runs move_matmul_waits_to_-
    # ldweights + generate_event_semaphores, which split semaphore waits to
    # satisfy the TRN2 one-wait-per-instruction constraint.
    nc = bacc.Bacc()

    h = nc.declare_dram_parameter("h", [NT, D_IN], f32, isOutput=False)
    cblob = nc.declare_dram_parameter("cblob", [P, CB], f32, isOutput=False)
    upw = nc.declare_dram_parameter("upw", [RANK, D_OUT], f32, isOutput=False)
    out = nc.declare_dram_parameter("out", [NT, D_OUT], f32, isOutput=True)

    eq = mybir.AluOpType.is_equal
    mult = mybir.AluOpType.mult

    with tile.TileContext(nc) as tc:
        with (
            tc.tile_pool(name="const", bufs=1) as const,
            tc.tile_pool(name="hnat", bufs=3) as hnat_pool,
            tc.tile_pool(name="hT", bufs=2) as hT_pool,
            tc.tile_pool(name="mask", bufs=4) as mask_pool,
            tc.tile_pool(name="resT", bufs=2) as resT_pool,
            tc.tile_pool(name="outsb", bufs=2) as out_pool,
            tc.tile_pool(name="psum_h", bufs=2, space="PSUM") as psum_h_pool,
            tc.tile_pool(name="psum_dn", bufs=2, space="PSUM") as psum_dn_pool,
            tc.tile_pool(name="psum_up", bufs=2, space="PSUM") as psum_up_pool,
            tc.tile_pool(name="psum_trm", bufs=2, space="PSUM") as psum_trm_pool,
        ):
            cb = const.tile([P, CB], f32)
            upT = const.tile([RANK, D_OUT], f32)

            nc.sync.dma_start(out=cb[:], in_=cblob[:, :])
            nc.sync.dma_start(out=upT[:], in_=upw[:, :])

            # f32r operands must come from a rounding producer; DMA can't
            # round, so copy the weights into f32r tiles once.
            dwT_r = const.tile([P, NKC * RANK], f32r)
            upT_r = const.tile([RANK, D_OUT], f32r)
            nc.vector.tensor_copy(out=dwT_r[:], in_=cb[:, C_DWT:C_DWT + NKC * RANK])
            nc.scalar.copy(out=upT_r[:], in_=upT[:])

            dwT = cb[:, C_DWT:C_DWT + NKC * RANK]
            ident = cb[:, C_ID:C_ID + P]
            iota_sb = cb[:, C_IOTA:C_IOTA + RANK]
            idx_sb = cb[:, C_IDX:C_IDX + NCHUNK * TOPK]
            val_sb = cb[:, C_VAL:C_VAL + NCHUNK * TOPK]

            copy_engines = [nc.vector.tensor_copy, nc.scalar.copy]
            cp_i = 0

            for tt in range(NTILES):
                # 1. natural-layout loads, one per 128-token chunk
                h_nats = []
                for j in range(NJ):
                    h_nat = hnat_pool.tile([P, D_IN], f32)
                    row = tt * TT + j * P
                    nc.sync.dma_start(out=h_nat[:], in_=h[row:row + P, :])
                    h_nats.append(h_nat)

                # 2. PE-transpose h blocks into hT; 4 transposes (2 ki x 2 j)
                #    share one PSUM bank -> one fat [128, 512] copy
                hT = hT_pool.tile([P, NKC * TT], f32r)
                for kb in range(NKC // 2):
                    psum_h = psum_h_pool.tile([P, 2 * TT], f32)
                    for ki2 in range(2):
                        ki = kb * 2 + ki2
                        for j in range(NJ):
                            nc.tensor.transpose(
                                psum_h[:, ki2 * TT + j * P:ki2 * TT + (j + 1) * P],
                                h_nats[j][:, ki * P:(ki + 1) * P],
                                ident[:],
                            )
                    cp = copy_engines[cp_i % 2]
                    cp_i += 1
                    cp(
                        out=hT[:, kb * 2 * TT:(kb + 1) * 2 * TT],
                        in_=psum_h[:],
                    )

                # 3. down projection, accumulated over NKC chunks (f32r)
                psum_dn = psum_dn_pool.tile([RANK, TT], f32)
                for ki in range(NKC):
                    nc.tensor.matmul(
                        psum_dn[:],
                        lhsT=dwT_r[:, ki * RANK:(ki + 1) * RANK],
                        rhs=hT[:, ki * TT:(ki + 1) * TT],
                        start=(ki == 0),
                        stop=(ki == NKC - 1),
                    )

                # psum_dn -> SBUF so the mask multiply has one PSUM operand
                down_sb = resT_pool.tile([RANK, TT], f32, tag="down_sb")
                nc.scalar.copy(out=down_sb[:], in_=psum_dn[:])

                resT = resT_pool.tile([RANK, TT], f32r)
                for j in range(NJ):
                    jj = tt * NJ + j
                    # 4. top-k scatter mask: one-hot*val per k on DVE, summed
                    #    in PSUM via accumulating transpose matmuls
                    psum_tr = psum_trm_pool.tile([RANK, P], f32)
                    for k in range(TOPK):
                        col = jj * TOPK + k
                        oh = mask_pool.tile([P, RANK], f32)
                        nc.vector.tensor_scalar(
                            out=oh[:],
                            in0=iota_sb[:],
                            scalar1=idx_sb[:, col:col + 1],
                            scalar2=val_sb[:, col:col + 1],
                            op0=eq,
                            op1=mult,
                        )
                        nc.tensor.matmul(
                            psum_tr[:],
                            lhsT=oh[:],
                            rhs=ident[:],
                            is_transpose=True,
                            start=(k == 0),
                            stop=(k == TOPK - 1),
                        )
                    nc.vector.tensor_mul(
                        resT[:, j * P:(j + 1) * P],
                        down_sb[:, j * P:(j + 1) * P],
                        psum_tr[:],
                    )

                    # 5. up projection (f32r) + fat store
                    out_sb = out_pool.tile([P, D_OUT], f32)
                    for o in range(NOT):
                        psum_up = psum_up_pool.tile([P, OT], f32)
                        nc.tensor.matmul(
                            psum_up[:],
                            lhsT=resT[:, j * P:(j + 1) * P],
                            rhs=upT_r[:, o * OT:(o + 1) * OT],
                            start=True,
                            stop=True,
                        )
                        cp = copy_engines[cp_i % 2]
                        cp_i += 1
                        cp(
                            out=out_sb[:, o * OT:(o + 1) * OT],
                            in_=psum_up[:],
                        )
                    nc.sync.dma_start(
                        out=out[jj * P:(jj + 1) * P, :],
                        in_=out_sb[:],
                    )

    # Run the Bacc pipeline (register alloc + wait splitting for the TRN2
    # one-wait-per-instruction constraint) before the module is serialized.
    nc.finalize()
    return nc


def _get_program():
    if "nc" not in _CACHE:
        _CACHE["nc"] = _build_program()
    return _CACHE["nc"]


def prepare_in_maps(hidden_states, down_w, up_w, top_k_values, top_k_indices):
    h = np.ascontiguousarray(hidden_states, dtype=np.float32)
    dw = np.ascontiguousarray(down_w, dtype=np.float32)
    uw = np.ascontiguousarray(up_w, dtype=np.float32)
    vals = np.ascontiguousarray(top_k_values, dtype=np.float32)
    idxf = top_k_indices.astype(np.float32)

    upT = np.ascontiguousarray(uw.T)  # [64, 4096]

    # dwT[i, kc*64 + r] = dw[r, kc*128 + i]
    dwT = dw.reshape(RANK, NKC, P).transpose(2, 1, 0).reshape(P, NKC * RANK)
    ident = np.eye(P, dtype=np.float32)
    iota = np.broadcast_to(np.arange(RANK, dtype=np.float32), (P, RANK))

    in_maps = []
    for c in range(NCORES):
        s = slice(c * NT, (c + 1) * NT)
        # idx/val packed [p, chunk*8 + k] for this core's 16 chunks
        idx_p = idxf[s].reshape(NCHUNK, P, TOPK).transpose(1, 0, 2).reshape(P, -1)
        val_p = vals[s].reshape(NCHUNK, P, TOPK).transpose(1, 0, 2).reshape(P, -1)
        cb = np.concatenate([dwT, ident, iota, idx_p, val_p], axis=1)
        assert cb.shape == (P, CB)
        in_maps.append(
            {
                "h": h[s],
                "cblob": np.ascontiguousarray(cb),
                "upw": upT,
            }
        )
    return in_maps


def kernel(hidden_states, down_w, up_w, top_k_values, top_k_indices, **_kw):
    from concourse.bass_utils import run_bass_kernel_spmd

    nc = _get_program()
    in_maps = prepare_in_maps(
        hidden_states, down_w, up_w, top_k_values, top_k_indices
    )
    res = run_bass_kernel_spmd(nc, in_maps, core_ids=list(range(NCORES)))
    return np.concatenate([r["out"] for r in res.results], axis=0)



# revision 2
# speedup vs baseline: 1.8422x; 1.8422x over previous
"""MoE LoRA linear layer kernel for Trainium2, data-parallel over 8 NeuronCores.

Math (per token n):
    down = h @ down_w.T                      [N, 64]
    mask[n, r] = val[n, k] if idx[n, k] == r else 0   (indices distinct per row)
    out = (down * mask) @ up_w.T             [N, 4096]

Sharding: tokens split 8 ways (2048/core); LoRA weights replicated.

Key layout decisions (all host-side prep; HW does both matmuls + masking):
  - h is transposed + bf16-cast on the host and packed so each token tile
    is one fully contiguous 4 MiB DMA ([128, 32*512] per tile).  This
    removes the 512 on-device PE transposes + 8.4M elements of PSUM->SBUF
    copies the natural-layout path needs.
  - The top-k scatter (idx/val -> dense [64, NT] maskT) is host-packed like
    the baseline's idx/val chunk repack; the value multiply happens on
    device (DVE) against the down-projection PSUM result.
  - Output is stored bf16 (2e-2 rel-err budget; bf16 ~4e-3) and upcast on
    the host, halving store traffic.

Per-core pipeline (4 token tiles of 512):
  1. one 4 MiB contiguous DMA loads hT tile [128, 32*512] bf16
  2. 32 accumulating bf16 matmuls -> psum_dn [64, 512]
  3. resT = psum_dn * maskT slice (DVE, bf16 out)
  4. per 128-token chunk: 8 matmuls [K=64, N=512] -> psum, DVE/ACT copy
     (bf16 downcast) into out_sb [128, 4096], one 1 MiB store per chunk

HBM traffic per core: 16 MiB in + 16 MiB out + ~1.3 MiB weights ~= 33 MiB,
vs ~50 MiB PE/DVE-bound work in the f32 natural-layout baseline.
"""

import sys

for p in ("/opt/trn_rl_repo", "/opt/pypackages"):
    if p not in sys.path:
        sys.path.insert(0, p)

import numpy as np

N, D_IN, D_OUT, RANK, TOPK = 16384, 4096, 4096, 64, 8
NCORES = 8
NT = N // NCORES          # tokens per core = 2048
P = 128                   # partitions
TT = 512                  # token tile (down-matmul free dim, one PSUM bank)
NKC = D_IN // P           # 32 contraction chunks for down proj
NTILES = NT // TT         # 4 token tiles per core
NJ = TT // P              # 4 x 128-token chunks per tile
OT = 512                  # output col tile (one PSUM bank)
NOT = D_OUT // OT         # 8 output col tiles

_CACHE = {}


def _build_program():
    import concourse.bacc as bacc
    import concourse.mybir as mybir
    from concourse import tile

    f32 = mybir.dt.float32
    bf16 = mybir.dt.bfloat16
    # Bacc (not plain Bass): its finalize() runs move_matmul_waits_to_-
    # ldweights + generate_event_semaphores, which split semaphore waits to
    # satisfy the TRN2 one-wait-per-instruction constraint.
    nc = bacc.Bacc()

    ht = nc.declare_dram_parameter("ht", [NTILES * P, NKC * TT], bf16, isOutput=False)
    dwt = nc.declare_dram_parameter("dwt", [P, NKC * RANK], bf16, isOutput=False)
    upw = nc.declare_dram_parameter("upw", [RANK, D_OUT], bf16, isOutput=False)
    maskt = nc.declare_dram_parameter("maskt", [RANK, NT], f32, isOutput=False)
    out = nc.declare_dram_parameter("out", [NT, D_OUT], bf16, isOutput=True)

    with tile.TileContext(nc) as tc:
        with (
            tc.tile_pool(name="const", bufs=1) as const,
            tc.tile_pool(name="ht", bufs=2) as ht_pool,
            tc.tile_pool(name="resT", bufs=2) as resT_pool,
            tc.tile_pool(name="outsb", bufs=4) as out_pool,
            tc.tile_pool(name="psum_dn", bufs=2, space="PSUM") as psum_dn_pool,
            tc.tile_pool(name="psum_up", bufs=4, space="PSUM") as psum_up_pool,
        ):
            dwt_sb = const.tile([P, NKC * RANK], bf16)
            upT_sb = const.tile([RANK, D_OUT], bf16)
            maskT_sb = const.tile([RANK, NT], f32)
            nc.sync.dma_start(out=dwt_sb[:], in_=dwt[:, :])
            nc.sync.dma_start(out=upT_sb[:], in_=upw[:, :])
            nc.sync.dma_start(out=maskT_sb[:], in_=maskt[:, :])

            copy_engines = [nc.vector.tensor_copy, nc.scalar.copy]
            cp_i = 0

            for tt in range(NTILES):
                ht_sb = ht_pool.tile([P, NKC * TT], bf16)
                nc.sync.dma_start(out=ht_sb[:], in_=ht[tt * P:(tt + 1) * P, :])

                psum_dn = psum_dn_pool.tile([RANK, TT], f32)
                for ki in range(NKC):
                    nc.tensor.matmul(
                        psum_dn[:],
                        lhsT=dwt_sb[:, ki * RANK:(ki + 1) * RANK],
                        rhs=ht_sb[:, ki * TT:(ki + 1) * TT],
                        start=(ki == 0),
                        stop=(ki == NKC - 1),
                    )

                resT = resT_pool.tile([RANK, TT], bf16)
                nc.vector.tensor_mul(
                    resT[:],
                    maskT_sb[:, tt * TT:(tt + 1) * TT],
                    psum_dn[:],
                )

                for j in range(NJ):
                    out_sb = out_pool.tile([P, D_OUT], bf16)
                    for o in range(NOT):
                        psum_up = psum_up_pool.tile([P, OT], f32)
                        nc.tensor.matmul(
                            psum_up[:],
                            lhsT=resT[:, j * P:(j + 1) * P],
                            rhs=upT_sb[:, o * OT:(o + 1) * OT],
                            start=True,
                            stop=True,
                        )
                        cp = copy_engines[cp_i % 2]
                        cp_i += 1
                        cp(out=out_sb[:, o * OT:(o + 1) * OT], in_=psum_up[:])
                    row = (tt * NJ + j) * P
                    nc.sync.dma_start(out=out[row:row + P, :], in_=out_sb[:])

    nc.finalize()
    return nc


def _get_program():
    if "nc" not in _CACHE:
        _CACHE["nc"] = _build_program()
    return _CACHE["nc"]


def prepare_in_maps(hidden_states, down_w, up_w, top_k_values, top_k_indices):
    import ml_dtypes

    bf16 = ml_dtypes.bfloat16

    h = np.asarray(hidden_states, dtype=np.float32)
    dw = np.asarray(down_w, dtype=np.float32)
    uw = np.asarray(up_w, dtype=np.float32)
    vals = np.asarray(top_k_values, dtype=np.float32)
    idx = np.asarray(top_k_indices).astype(np.int64)

    # dwT[p, ki*64 + r] = dw[r, ki*128 + p]
    dwT = np.ascontiguousarray(
        dw.reshape(RANK, NKC, P).transpose(2, 1, 0).reshape(P, NKC * RANK)
    ).astype(bf16)
    upT = np.ascontiguousarray(uw.T).astype(bf16)  # [64, 4096]

    # dense scatter of top-k values: mask[n, r] = val[n, k] where idx[n,k]==r
    mask = np.zeros((N, RANK), dtype=np.float32)
    rows = np.arange(N)[:, None]
    mask[rows, idx] = vals

    in_maps = []
    for c in range(NCORES):
        s = slice(c * NT, (c + 1) * NT)
        # ht[tt*128 + p, ki*512 + u] = h[c*NT + tt*512 + u, ki*128 + p]
        ht = (
            h[s]
            .reshape(NTILES, TT, NKC, P)
            .transpose(0, 3, 2, 1)
            .reshape(NTILES * P, NKC * TT)
            .astype(bf16)
        )
        maskT = np.ascontiguousarray(mask[s].T)  # [64, 2048] f32
        in_maps.append(
            {
                "ht": np.ascontiguousarray(ht),
                "dwt": dwT,
                "upw": upT,
                "maskt": maskT,
            }
        )
    return in_maps


def kernel(hidden_states, down_w, up_w, top_k_values, top_k_indices, **_kw):
    from concourse.bass_utils import run_bass_kernel_spmd

    nc = _get_program()
    in_maps = prepare_in_maps(
        hidden_states, down_w, up_w, top_k_values, top_k_indices
    )
    res = run_bass_kernel_spmd(nc, in_maps, core_ids=list(range(NCORES)))
    return np.concatenate(
        [np.asarray(r["out"], dtype=np.float32) for r in res.results], axis=0
    )


# revision 4
# speedup vs baseline: 2.3827x; 1.2934x over previous
"""MoE LoRA linear layer kernel for Trainium2, data-parallel over 8 NeuronCores.

Math (per token n):
    down = h @ down_w.T                      [N, 64]
    mask[n, r] = val[n, k] if idx[n, k] == r else 0   (indices distinct per row)
    out = (down * mask) @ up_w.T             [N, 4096]

Sharding: tokens split 8 ways (2048/core); LoRA weights replicated.

Key layout decisions (all host-side prep; HW does both matmuls + masking):
  - h is transposed + bf16-cast on the host and packed so each token tile
    is one fully contiguous 4 MiB DMA ([128, 32*512] per tile).  This
    removes the 512 on-device PE transposes + 8.4M elements of PSUM->SBUF
    copies the natural-layout path needs.
  - The top-k scatter (idx/val -> dense [64, NT] maskT) is host-packed like
    the baseline's idx/val chunk repack; the value multiply happens on
    device (DVE) against the down-projection PSUM result.
  - Output is stored bf16 (2e-2 rel-err budget; bf16 ~4e-3) and upcast on
    the host, halving store traffic.

Per-core pipeline (4 token tiles of 512):
  1. one 4 MiB contiguous DMA loads hT tile [128, 32*512] bf16
  2. 32 accumulating bf16 matmuls -> psum_dn [64, 512]
  3. resT = psum_dn * maskT slice (DVE, bf16 out)
  4. per 128-token chunk: 8 matmuls [K=64, N=512] -> psum, DVE/ACT copy
     (bf16 downcast) into out_sb [128, 4096], one 1 MiB store per chunk

HBM traffic per core: 16 MiB in + 16 MiB out + ~1.3 MiB weights ~= 33 MiB,
vs ~50 MiB PE/DVE-bound work in the f32 natural-layout baseline.
"""

import sys

for p in ("/opt/trn_rl_repo", "/opt/pypackages"):
    if p not in sys.path:
        sys.path.insert(0, p)

import numpy as np

N, D_IN, D_OUT, RANK, TOPK = 16384, 4096, 4096, 64, 8
NCORES = 8
NT = N // NCORES          # tokens per core = 2048
P = 128                   # partitions
TT = 512                  # token tile (down-matmul free dim, one PSUM bank)
NKC = D_IN // P           # 32 contraction chunks for down proj
NTILES = NT // TT         # 4 token tiles per core
NJ = TT // P              # 4 x 128-token chunks per tile
OT = 512                  # output col tile (one PSUM bank)
NOT = D_OUT // OT         # 8 output col tiles

_CACHE = {}


def _build_program():
    import concourse.bacc as bacc
    import concourse.mybir as mybir
    from concourse import tile

    f32 = mybir.dt.float32
    bf16 = mybir.dt.bfloat16
    # Bacc (not plain Bass): its finalize() runs move_matmul_waits_to_-
    # ldweights + generate_event_semaphores, which split semaphore waits to
    # satisfy the TRN2 one-wait-per-instruction constraint.
    nc = bacc.Bacc()

    ht = nc.declare_dram_parameter("ht", [NTILES * P, NKC * TT], bf16, isOutput=False)
    dwt = nc.declare_dram_parameter("dwt", [P, NKC * RANK], bf16, isOutput=False)
    upw = nc.declare_dram_parameter("upw", [RANK, D_OUT], bf16, isOutput=False)
    maskt = nc.declare_dram_parameter("maskt", [RANK, NT], f32, isOutput=False)
    out = nc.declare_dram_parameter("out", [NT, D_OUT], bf16, isOutput=True)

    HK = NKC // 2  # ki chunks per ht half-tile

    with tile.TileContext(nc) as tc:
        with (
            tc.tile_pool(name="const", bufs=1) as const,
            tc.tile_pool(name="ht", bufs=6) as ht_pool,
            tc.tile_pool(name="resT", bufs=2) as resT_pool,
            tc.tile_pool(name="outsb", bufs=4) as out_pool,
            tc.tile_pool(name="psum_dn", bufs=2, space="PSUM") as psum_dn_pool,
            tc.tile_pool(name="psum_up", bufs=4, space="PSUM") as psum_up_pool,
        ):
            dwt_sb = const.tile([P, NKC * RANK], bf16)
            upT_sb = const.tile([RANK, D_OUT], bf16)
            maskT_sb = const.tile([RANK, NT], f32)
            # dwt first (gates the first matmul), then ht0 is issued by the
            # loop below; upw/maskt aren't needed until the first up-proj.
            nc.sync.dma_start(out=dwt_sb[:], in_=dwt[:, :])

            copy_engines = [nc.vector.tensor_copy, nc.scalar.copy]
            # DVE copies are ~1.8x faster than ACT; give DVE 5 of every 8.
            copy_pick = [0, 1, 0, 1, 0, 1, 0, 0]

            for tt in range(NTILES):
                # ht tile split in two halves so down matmuls start after
                # 2 MiB instead of 4 MiB of DMA.
                ht_halves = []
                for hh in range(2):
                    ht_sb = ht_pool.tile([P, HK * TT], bf16)
                    nc.sync.dma_start(
                        out=ht_sb[:],
                        in_=ht[tt * P:(tt + 1) * P,
                               hh * HK * TT:(hh + 1) * HK * TT],
                    )
                    ht_halves.append(ht_sb)
                if tt == 0:
                    nc.sync.dma_start(out=upT_sb[:], in_=upw[:, :])
                    nc.sync.dma_start(out=maskT_sb[:], in_=maskt[:, :])

                psum_dn = psum_dn_pool.tile([RANK, TT], f32)
                for ki in range(NKC):
                    nc.tensor.matmul(
                        psum_dn[:],
                        lhsT=dwt_sb[:, ki * RANK:(ki + 1) * RANK],
                        rhs=ht_halves[ki // HK][:, (ki % HK) * TT:(ki % HK + 1) * TT],
                        start=(ki == 0),
                        stop=(ki == NKC - 1),
                    )

                resT = resT_pool.tile([RANK, TT], bf16)
                nc.vector.tensor_mul(
                    resT[:],
                    maskT_sb[:, tt * TT:(tt + 1) * TT],
                    psum_dn[:],
                )

                for j in range(NJ):
                    out_sb = out_pool.tile([P, D_OUT], bf16)
                    for o in range(NOT):
                        psum_up = psum_up_pool.tile([P, OT], f32)
                        nc.tensor.matmul(
                            psum_up[:],
                            lhsT=resT[:, j * P:(j + 1) * P],
                            rhs=upT_sb[:, o * OT:(o + 1) * OT],
                            start=True,
                            stop=True,
                        )
                        cp = copy_engines[copy_pick[o]]
                        cp(out=out_sb[:, o * OT:(o + 1) * OT], in_=psum_up[:])
                    row = (tt * NJ + j) * P
                    # SWDGE queue: keeps store waits off the Sync queue so
                    # they can't head-of-line-block the next ht load.
                    nc.gpsimd.dma_start(out=out[row:row + P, :], in_=out_sb[:])

    nc.finalize()
    return nc


def _get_program():
    if "nc" not in _CACHE:
        _CACHE["nc"] = _build_program()
    return _CACHE["nc"]


def prepare_in_maps(hidden_states, down_w, up_w, top_k_values, top_k_indices):
    import ml_dtypes

    bf16 = ml_dtypes.bfloat16

    h = np.asarray(hidden_states, dtype=np.float32)
    dw = np.asarray(down_w, dtype=np.float32)
    uw = np.asarray(up_w, dtype=np.float32)
    vals = np.asarray(top_k_values, dtype=np.float32)
    idx = np.asarray(top_k_indices).astype(np.int64)

    # dwT[p, ki*64 + r] = dw[r, ki*128 + p]
    dwT = np.ascontiguousarray(
        dw.reshape(RANK, NKC, P).transpose(2, 1, 0).reshape(P, NKC * RANK)
    ).astype(bf16)
    upT = np.ascontiguousarray(uw.T).astype(bf16)  # [64, 4096]

    # dense scatter of top-k values: mask[n, r] = val[n, k] where idx[n,k]==r
    mask = np.zeros((N, RANK), dtype=np.float32)
    rows = np.arange(N)[:, None]
    mask[rows, idx] = vals

    in_maps = []
    for c in range(NCORES):
        s = slice(c * NT, (c + 1) * NT)
        # ht[tt*128 + p, ki*512 + u] = h[c*NT + tt*512 + u, ki*128 + p]
        ht = (
            h[s]
            .reshape(NTILES, TT, NKC, P)
            .transpose(0, 3, 2, 1)
            .reshape(NTILES * P, NKC * TT)
            .astype(bf16)
        )
        maskT = np.ascontiguousarray(mask[s].T)  # [64, 2048] f32
        in_maps.append(
            {
                "ht": np.ascontiguousarray(ht),
                "dwt": dwT,
                "upw": upT,
                "maskt": maskT,
            }
        )
    return in_maps


def kernel(hidden_states, down_w, up_w, top_k_values, top_k_indices, **_kw):
    from concourse.bass_utils import run_bass_kernel_spmd

    nc = _get_program()
    in_maps = prepare_in_maps(
        hidden_states, down_w, up_w, top_k_values, top_k_indices
    )
    res = run_bass_kernel_spmd(nc, in_maps, core_ids=list(range(NCORES)))
    return np.concatenate(
        [np.asarray(r["out"], dtype=np.float32) for r in res.results], axis=0
    )
